# revision 1
# baseline (speedup 1.0000x reference)
"""Trainium2 Bass kernel: causal multi-head attention block (B=2, T=4096, C=768, H=12).

Sharding: 8 cores = 2 batches x 4 head-groups (3 heads each). Each core runs
QKV projection + causal flash attention + a partial output projection over its
3 heads' channels. Host sums the 4 partials per batch and adds b_proj.

Attention is transpose-free: scores are computed as scores^T[key, query] via
matmul(lhsT=kT, rhs=qT) with both q and k produced in [head_size, T] layout by
the QKV matmul (host-pretransposed weights). Softmax denominators come from a
65th all-ones column in the AV matmul's stationary operand; normalization uses
reciprocal + gpsimd partition-broadcast. K=64 score matmuls are row-packed in
pairs on the 128x128 PE array: heads 0/1 share a q-block; the lone head 2 packs
two adjacent q-blocks.
"""

import sys

for p in ("/opt/trn_rl_repo",):
    if p not in sys.path:
        sys.path.insert(0, p)

from contextlib import ExitStack

import ml_dtypes
import numpy as np

import concourse.bass as bass  # noqa: F401
import concourse.mybir as mybir
import concourse.tile as tile
from concourse import bacc
from concourse.bass_utils import run_bass_kernel_spmd

BF16 = ml_dtypes.bfloat16
F32 = np.float32

B, T, C = 2, 4096, 768
NH, HS = 12, 64
SCALE = HS**-0.5
HPC = 3  # heads per core
N_CORES = 8
P = 128
TQ = 512  # query block width
NQP = T // (2 * TQ)  # q-block pairs = 4
CCH = C // P  # contraction chunks over embed dim = 6
NV = HPC * (HS + 1)  # v columns incl. ones = 195

_DT_BF16 = mybir.dt.bfloat16
_DT_F32 = mybir.dt.float32

_NC = None  # cached compiled Bass module


def _build_bass(repeat=1, loop_reps=0):
    nc = bacc.Bacc("TRN2", target_bir_lowering=False)

    xT_d = nc.dram_tensor("xT", [C, T], _DT_BF16, kind="ExternalInput")
    wqk_d = nc.dram_tensor("wqk", [C, 512], _DT_BF16, kind="ExternalInput")
    wv_d = nc.dram_tensor("wv", [C, NV], _DT_BF16, kind="ExternalInput")
    wproj_d = nc.dram_tensor("wproj", [256, C], _DT_BF16, kind="ExternalInput")
    bqk_d = nc.dram_tensor("bqk", [P, 4], _DT_F32, kind="ExternalInput")
    bv_d = nc.dram_tensor("bv", [P, NV], _DT_F32, kind="ExternalInput")
    masks_d = nc.dram_tensor("masks", [P, 4, TQ], _DT_BF16, kind="ExternalInput")
    out_d = nc.dram_tensor("out", [T, C], _DT_F32, kind="ExternalOutput")

    EXP = mybir.ActivationFunctionType.Exp
    MULT = mybir.AluOpType.mult
    BYPASS = mybir.AluOpType.bypass

    with tile.TileContext(nc) as tc, ExitStack() as ctx:
        const = ctx.enter_context(tc.tile_pool(name="const", bufs=1))
        sb = ctx.enter_context(tc.tile_pool(name="work_sb", bufs=4))
        sbd = ctx.enter_context(tc.tile_pool(name="stage_sb", bufs=6))

        xT_sb = const.tile([P, CCH, T], _DT_BF16, tag="xT", name="xT")
        wqk_sb = const.tile([P, CCH, 512], _DT_BF16, tag="wqk", name="wqk")
        wv_sb = const.tile([P, CCH, NV], _DT_BF16, tag="wv", name="wv")
        wproj_sb = const.tile([P, 2, C], _DT_BF16, tag="wproj", name="wproj")
        bqk_sb = const.tile([P, 4], _DT_F32, tag="bqk", name="bqk")
        bv_sb = const.tile([P, NV], _DT_F32, tag="bv", name="bv")
        masks_sb = const.tile([P, 4, TQ], _DT_BF16, tag="masks", name="masks")
        # qT/kT chunks: 0 = [h0 ; h1], 1 = [h2 ; h2]
        qT_sb = const.tile([P, 2, T], _DT_BF16, tag="qT", name="qT")
        kT_sb = const.tile([P, 2, T], _DT_BF16, tag="kT", name="kT")
        v_sb = const.tile([P, T // P, NV], _DT_BF16, tag="v", name="v")
        aoT_sb = const.tile([P, 2, T], _DT_BF16, tag="aoT", name="aoT")

        xT_r = xT_d[:].rearrange("(c p) t -> p c t", p=P)
        nc.sync.dma_start(wqk_sb[:], wqk_d[:].rearrange("(c p) m -> p c m", p=P))
        nc.sync.dma_start(wv_sb[:], wv_d[:].rearrange("(c p) m -> p c m", p=P))
        nc.sync.dma_start(wproj_sb[:], wproj_d[:].rearrange("(k p) n -> p k n", p=P))
        nc.sync.dma_start(bqk_sb[:], bqk_d[:])
        nc.sync.dma_start(bv_sb[:], bv_d[:])
        nc.sync.dma_start(masks_sb[:], masks_d[:])

        # ---- single interleaved pipeline: qkv tiles -> attention pair -> proj
        if loop_reps:
            with tc.For_i(0, loop_reps, 1):
                _emit_pipeline(nc, tc, sb, locals())
        else:
            for _rep in range(repeat):
                _emit_pipeline(nc, tc, sb, locals())

    nc.compile()
    return nc


def _emit_pipeline(nc, tc, sb, env):
    sbd = env["sbd"]
    xT_sb = env["xT_sb"]; wqk_sb = env["wqk_sb"]; wv_sb = env["wv_sb"]
    wproj_sb = env["wproj_sb"]; bqk_sb = env["bqk_sb"]; bv_sb = env["bv_sb"]
    masks_sb = env["masks_sb"]; qT_sb = env["qT_sb"]; kT_sb = env["kT_sb"]
    v_sb = env["v_sb"]; aoT_sb = env["aoT_sb"]; xT_r = env["xT_r"]
    out_d = env["out_d"]
    EXP = env["EXP"]; MULT = env["MULT"]; BYPASS = env["BYPASS"]
    if True:
        with (
            tc.tile_pool(name="sc_ps", bufs=2, space="PSUM") as sc_ps,
            tc.tile_pool(name="av_ps", bufs=2, space="PSUM") as av_ps,
            tc.tile_pool(name="mm_ps", bufs=2, space="PSUM") as mm_ps,
        ):

            def emit_qkv_tile(tt):
                tsl = slice(tt * TQ, (tt + 1) * TQ)
                for c in range(CCH):
                    nc.sync.dma_start(xT_sb[:, c, tsl], xT_r[:, c, tsl])
                for m in range(4):
                    ps = mm_ps.tile([P, TQ], _DT_F32, tag="small", name="qkps")
                    for c in range(CCH):
                        nc.tensor.matmul(
                            ps,
                            wqk_sb[:, c, m * P : (m + 1) * P],
                            xT_sb[:, c, tsl],
                            start=(c == 0),
                            stop=(c == CCH - 1),
                        )
                    dst = qT_sb if m < 2 else kT_sb
                    nc.scalar.activation(
                        dst[:, m % 2, tsl],
                        ps,
                        mybir.ActivationFunctionType.Identity,
                        bias=bqk_sb[:, m : m + 1],
                    )
                for t4 in range(TQ // P):
                    tch = tt * (TQ // P) + t4
                    psv = mm_ps.tile([P, TQ], _DT_F32, tag="small", name="vps")
                    psv = psv[:, :NV]
                    for c in range(CCH):
                        nc.tensor.matmul(
                            psv,
                            xT_sb[:, c, tch * P : (tch + 1) * P],
                            wv_sb[:, c, :],
                            start=(c == 0),
                            stop=(c == CCH - 1),
                        )
                    nc.vector.tensor_add(v_sb[:, tch], psv, bv_sb[:])

            def normalize(avt, h, tq0):
                """aoT[.., tq0:tq0+TQ] = avt[0:64] / avt[64] (denominator row)."""
                rc = sb.tile([1, TQ], _DT_F32, tag="rc", name="rc")
                nc.vector.reciprocal(rc, avt[64:65, :])
                bcs = sb.tile([64, TQ], _DT_F32, tag="bcs", name="bcs")
                nc.gpsimd.partition_broadcast(bcs[:], rc[:], channels=64)
                dsts = {0: ((0, 0),), 1: ((0, 64),), 2: ((1, 0), (1, 64))}[h]
                for chunk, r0 in dsts:
                    nc.vector.scalar_tensor_tensor(
                        out=aoT_sb[r0 : r0 + 64, chunk, tq0 : tq0 + TQ],
                        in0=avt[0:64, :],
                        scalar=1.0,
                        in1=bcs[:],
                        op0=BYPASS,
                        op1=MULT,
                    )

            def attend(hrows, hchunk, vcol0, tq_lo, tq_hi, n_lo, n_hi, h, diag_only):
                """Row-packed pair of score streams + av accumulation.

                Stream 'lo' uses kT/qT rows 0:64 (q-block tq_lo, n_lo chunks);
                stream 'hi' uses rows 64:128 (q-block tq_hi, n_hi chunks).
                hrows: (head of lo stream, head of hi stream) for v columns.
                """
                av_lo = av_ps.tile([65, TQ], _DT_F32, tag="av0", name="av0")
                av_hi = av_ps.tile([65, TQ], _DT_F32, tag="av1", name="av1")
                for cg in range(0, n_hi, 2):
                    lo_act = cg < n_lo
                    st_lo = (
                        sbd.tile([P, 2, TQ], _DT_F32, tag="st", name="st_lo")
                        if lo_act
                        else None
                    )
                    st_hi = sbd.tile([P, 2, TQ], _DT_F32, tag="st", name="st_hi")
                    for j in (0, 1):
                        c = cg + j
                        ksl = slice(c * P, (c + 1) * P)
                        if lo_act:
                            sc_lo = sc_ps.tile([P, TQ], _DT_F32, tag="sc", name="sc_lo")
                            nc.tensor.matmul(
                                sc_lo,
                                kT_sb[0:64, hchunk, ksl],
                                qT_sb[0:64, hchunk, tq_lo : tq_lo + TQ],
                                start=True,
                                stop=True,
                            )
                            nc.vector.tensor_copy(out=st_lo[:, j], in_=sc_lo)
                        sc_hi = sc_ps.tile([P, TQ], _DT_F32, tag="sc", name="sc_hi")
                        nc.tensor.matmul(
                            sc_hi,
                            kT_sb[64:P, hchunk, ksl],
                            qT_sb[64:P, hchunk, tq_hi : tq_hi + TQ],
                            start=True,
                            stop=True,
                        )
                        nc.vector.tensor_copy(out=st_hi[:, j], in_=sc_hi)
                    if lo_act:
                        pr_lo = sbd.tile([P, 2, TQ], _DT_BF16, tag="pr", name="pr_lo")
                        nc.scalar.activation(pr_lo[:], st_lo[:], EXP, scale=SCALE)
                    pr_hi = sbd.tile([P, 2, TQ], _DT_BF16, tag="pr", name="pr_hi")
                    nc.scalar.activation(pr_hi[:], st_hi[:], EXP, scale=SCALE)
                    for j in (0, 1):
                        c = cg + j
                        if lo_act and c >= n_lo - 4:
                            nc.vector.tensor_mul(
                                pr_lo[:, j], pr_lo[:, j], masks_sb[:, c - (n_lo - 4)]
                            )
                        if c >= n_hi - 4:
                            nc.vector.tensor_mul(
                                pr_hi[:, j], pr_hi[:, j], masks_sb[:, c - (n_hi - 4)]
                            )
                        if lo_act:
                            nc.tensor.matmul(
                                av_lo,
                                v_sb[:, c, hrows[0] * 65 : hrows[0] * 65 + 65],
                                pr_lo[:, j],
                                start=(c == 0),
                                stop=(c == n_lo - 1),
                            )
                        nc.tensor.matmul(
                            av_hi,
                            v_sb[:, c, hrows[1] * 65 : hrows[1] * 65 + 65],
                            pr_hi[:, j],
                            start=(c == 0),
                            stop=(c == n_hi - 1),
                        )
                normalize(av_lo, hrows[0] if h is None else h, tq_lo)
                normalize(av_hi, hrows[1] if h is None else h, tq_hi)

            def emit_proj(lo, hi):
                for tt in range(lo, hi):
                    t0 = tt * P
                    for nn in range(2):
                        nsl = slice(nn * 384, (nn + 1) * 384)
                        pp = mm_ps.tile([P, TQ], _DT_F32, tag="small", name="pj")
                        pp = pp[:, :384]
                        for kc in range(2):
                            nc.tensor.matmul(
                                pp,
                                aoT_sb[:, kc, t0 : t0 + P],
                                wproj_sb[:, kc, nsl],
                                start=(kc == 0),
                                stop=(kc == 1),
                            )
                        ot = sb.tile([P, 384], _DT_F32, tag="ot", name="ot")
                        nc.vector.tensor_copy(out=ot, in_=pp)
                        nc.sync.dma_start(out_d[t0 : t0 + P, nsl], ot)

            emit_qkv_tile(0)
            for qp in range(NQP):
                qb0, qb1 = 2 * qp, 2 * qp + 1
                # heads 0/1 share each q-block (full packing)
                nch = 4 * (qb0 + 1)
                attend((0, 1), 0, 0, qb0 * TQ, qb0 * TQ, nch, nch, None, False)
                if qp > 0:
                    emit_proj(8 * (qp - 1), 8 * (qp - 1) + 4)
                emit_qkv_tile(qb1)
                nch = 4 * (qb1 + 1)
                attend((0, 1), 0, 0, qb1 * TQ, qb1 * TQ, nch, nch, None, False)
                if qp > 0:
                    emit_proj(8 * (qp - 1) + 4, 8 * qp)
                if qp < NQP - 1:
                    emit_qkv_tile(2 * qp + 2)
                # head 2: pack the two adjacent q-blocks
                attend(
                    (2, 2), 1, 2, qb0 * TQ, qb1 * TQ, 4 * (qb0 + 1), 4 * (qb1 + 1),
                    2, False,
                )
            emit_proj(8 * NQP - 8, 8 * NQP - 4)
            emit_proj(8 * NQP - 4, 8 * NQP)


def _get_nc():
    global _NC
    if _NC is None:
        _NC = _build_bass()
    return _NC


def _core_inputs(x, w_attn, b_attn, core):
    """Host-side shard prep for one core."""
    b, g = divmod(core, 4)
    heads = [HPC * g + i for i in range(HPC)]
    h0, h1, h2 = heads

    xT = np.ascontiguousarray(x[b].T).astype(BF16)  # [C, T]

    # wqk cols: [q_h0 q_h1 | q_h2 q_h2 | k_h0 k_h1 | k_h2 k_h2]
    def rows(base, h):
        return w_attn[base + h * HS : base + (h + 1) * HS]

    def bias(base, h):
        return b_attn[base + h * HS : base + (h + 1) * HS]

    blocks, bias_chunks = [], []
    for base in (0, C):  # q rows then k rows
        blocks += [rows(base, h0), rows(base, h1), rows(base, h2), rows(base, h2)]
        bias_chunks.append(np.concatenate([bias(base, h0), bias(base, h1)]))
        bias_chunks.append(np.concatenate([bias(base, h2), bias(base, h2)]))
    wqk = np.ascontiguousarray(np.concatenate(blocks, 0).T.astype(BF16))  # [C, 512]
    bqk = np.stack(bias_chunks, 1).astype(F32)  # [128, 4]

    # wv cols: per head [v_h (64) | zero]; bias has 1.0 in the ones slot
    vblocks = []
    bv = np.zeros(NV, F32)
    for i, h in enumerate(heads):
        vblocks += [rows(2 * C, h), np.zeros((1, C), w_attn.dtype)]
        bv[i * 65 : i * 65 + HS] = bias(2 * C, h)
        bv[i * 65 + HS] = 1.0
    wv = np.ascontiguousarray(np.concatenate(vblocks, 0).T.astype(BF16))  # [C, 195]
    bv = np.broadcast_to(bv, (P, NV)).astype(F32)

    return xT, wqk, bqk, wv, bv, heads, b


def _masks_arr():
    p = np.arange(P)[:, None]
    n = np.arange(TQ)[None, :]
    m = np.stack([(n >= p + j * P) for j in range(4)], 0)  # [4, 128, 512]
    return np.ascontiguousarray(m.transpose(1, 0, 2).astype(BF16))


def _prep_in_maps(x, w_attn, b_attn, w_proj):
    masks = _masks_arr()
    in_maps = []
    for core in range(N_CORES):
        xT, wqk, bqk, wv, bv, heads, b = _core_inputs(x, w_attn, b_attn, core)
        h0, h1, h2 = heads
        c0 = np.concatenate(
            [
                w_proj[:, h0 * HS : (h0 + 1) * HS].T,
                w_proj[:, h1 * HS : (h1 + 1) * HS].T,
            ],
            0,
        )  # [128, C]
        half = 0.5 * w_proj[:, h2 * HS : (h2 + 1) * HS].T
        c1 = np.concatenate([half, half], 0)  # [128, C]
        wproj = np.ascontiguousarray(np.concatenate([c0, c1], 0).astype(BF16))
        in_maps.append(
            {
                "xT": xT,
                "wqk": wqk,
                "wv": wv,
                "wproj": wproj,
                "bqk": bqk,
                "bv": bv,
                "masks": masks,
            }
        )
    return in_maps


def _run(inputs, trace=False, **kw):
    x = np.asarray(inputs["x"], F32)
    w_attn = np.asarray(inputs["w_attn"], F32)
    b_attn = np.asarray(inputs["b_attn"], F32)
    w_proj = np.asarray(inputs["w_proj"], F32)
    b_proj = np.asarray(inputs["b_proj"], F32)

    nc = _get_nc()
    in_maps = _prep_in_maps(x, w_attn, b_attn, w_proj)
    res = run_bass_kernel_spmd(
        nc, in_maps, core_ids=list(range(N_CORES)), trace=trace, **kw
    )
    out = np.zeros((B, T, C), F32)
    for core in range(N_CORES):
        out[core // 4] += res.results[core]["out"]
    out += b_proj
    return out, res


def kernel(**inputs):
    out, _ = _run(inputs)
    return out

